# revision 18
# baseline (speedup 1.0000x reference)
"""RNN-T Joiner kernel for 8 Trainium2 NeuronCores.

out[b,t,u,:] = tanh(enc[b,t,:] + pred[b,u,:]) @ W.T + b

Sharding: data-parallel over t (400 -> 50 per core). Each core:
  - DVE/GPSIMD: broadcast-add encT[:,t] + predT[:,u] (f32 -> f32; bf16
    outputs would halve the DVE tensor_tensor rate)
  - ACT: tanh f32 -> bf16 logit (the cast rides the activation)
  - PE:  psum[cells, v] += logitT[c, cells].T @ WT[c, v]  (bf16, N=512)
  - output pass per 4-tile psum quad (FD=2048 amortizes the 120-cyc
    PSUM read bubble), two alternating paths so ACT shares the load:
      1-pass: DVE tensor_tensor psum(f32)+bias -> osb bf16   (~2.4us)
      2-pass: ACT quad copy psum->osb bf16 (~1.9us), DVE bf16+bias4_bf
              in place (2x_1P, ~1.2us)
  - DMA: one 500-cell (512KB) store per quad on the sync HWDGE queue
  - host: upcast bf16 -> f32 during the gather

Engine budget per core (cycle-model + measured): PE 640 MM x ~216ns =
~138us critical path; DVE ~116us (20 one-pass quads, 20 bf16 bias adds,
38 producer adds @1.2us), GPSIMD ~97us (42 adds @2.3us), ACT ~119us
(80 tanh @1.02us + 20 quad copies @1.85us), DMA out bf16 20.5MB ~50us.
BLK=5 keeps the startup serial chain (consts DMA -> add -> tanh) ~8us.
"""

import sys

sys.path.insert(0, "/opt/trn_rl_repo")

import ml_dtypes
import numpy as np

import concourse.bass as bass
import concourse.bacc as bacc
import concourse.mybir as mybir
from concourse.tile import TileContext
from concourse.bass_utils import run_bass_kernel_spmd

B, T, U, C, V = 4, 400, 100, 512, 512
NCORES = 8
TS = T // NCORES  # 50 t per core
P = 128
CK = C // P  # 4 chunks of the contraction dim
CELLS = TS * U  # 5000 cells (t,u) per batch per core
BLK = 5  # logit blocks per batch
BCELLS = CELLS // BLK  # 1000
BT = TS // BLK  # 10 t per block
TILE = 125  # uniform cell tile (8 per block); m<128 costs no PE streaming
NT = BCELLS // TILE  # 8
NQ = NT // 2  # tiles per psum quad... actually 4 tiles per quad, 2 quads
F32 = mybir.dt.float32
BF16 = mybir.dt.bfloat16

# f32 consts blobs. cf0 is tiny (block-0 enc slice + batch-0 pred) so the
# first producer isn't gated on the big transfer; cfr has the rest.
E0_OFF = 0  # [ck, t] for blocks 0-1 (t = 0..19)
P0_OFF = E0_OFF + CK * 2 * BT  # [ck, u]
NCOL_0 = P0_OFF + CK * U  # 480
BIAS_OFF = 0
ENC_OFF = BIAS_OFF + V  # [ck, b, t] (all b)
PRED_OFF = ENC_OFF + CK * B * TS  # [ck, b-1, u] (b = 1..3)
NCOL_R = PRED_OFF + CK * (B - 1) * U  # 2512
# bf16 consts blob: just W, pre-transposed [p, ck, v]
NCOL_W = CK * V  # 2048

N_BLOCKS = B * BLK  # 20

# producer engine per block (one [P, 4*BT*U] mega add+tanh per block):
# D=DVE add + ACT tanh, G=GPSIMD add + ACT tanh, A=ACT fused per-t ops.
# Block 0 is additionally split per-ck across DVE/GPSIMD so the first
# matmul isn't gated on one engine's serial 4000-col chain.
# S = blocks 0-1 (split per-ck across DVE+GPSIMD for startup). No ACT-fused
# blocks: they shift 7.4us of GPSIMD work into ~11us of serial ACT work and
# GPSIMD is not the constraint.
_BLOCK_ENG = list("SSDGG" "GGDGG" "GDGGG" "GGDGG")


_cache = {}


def _build():
    # Bacc (not raw Bass): its compile() runs generate_event_semaphores,
    # which splits >1-wait sync conditions that walrus rejects.
    nc = bacc.Bacc("TRN2", target_bir_lowering=False, debug=False)
    cf0 = nc.declare_dram_parameter("cf0", [P, NCOL_0], F32, isOutput=False)
    cfr = nc.declare_dram_parameter("cfr", [P, NCOL_R], F32, isOutput=False)
    cw = nc.declare_dram_parameter("cw", [P, NCOL_W], BF16, isOutput=False)
    # partition-strided output layout: [b, p, blk, j, v] with cell =
    # j*TILE + p inside a block. Per-partition lines are 8KB contiguous but
    # partitions are 40KB apart in DRAM, which keeps the DGE from coalescing
    # descriptors across partitions (coalesced = only ~5 of 16 SDMA engines
    # active = 130GB/s output ceiling). Host permutes back during the upcast.
    out = nc.declare_dram_parameter(
        "out", [B, TILE, BLK, NT, V], BF16, isOutput=True
    )

    with TileContext(nc) as tc:
        with (
            tc.tile_pool(name="consts", bufs=1) as cpool,
            tc.tile_pool(name="arg", bufs=4) as arg_pool,
            tc.tile_pool(name="logit", bufs=4) as logit_pool,
            tc.tile_pool(name="osb", bufs=3) as out_pool,
            tc.tile_pool(name="psum", bufs=4, space="PSUM") as psum_pool,
        ):
            cs0 = cpool.tile([P, NCOL_0], F32, tag="cs0")
            csf = cpool.tile([P, NCOL_R], F32, tag="csf")
            csw = cpool.tile([P, NCOL_W], BF16, tag="csw")
            nc.sync.dma_start(out=cs0, in_=cf0.ap())
            nc.sync.dma_start(out=csw, in_=cw.ap())
            nc.sync.dma_start(out=csf, in_=cfr.ap())

            wview = csw[:].rearrange("p (ck v) -> p ck v", ck=CK)
            bias_f32 = csf[:, BIAS_OFF : BIAS_OFF + V]
            eview = csf[:, ENC_OFF : ENC_OFF + CK * B * TS].rearrange(
                "p (ck b t) -> p ck b t", ck=CK, b=B
            )
            pview = csf[:, PRED_OFF : PRED_OFF + CK * (B - 1) * U].rearrange(
                "p (ck b u) -> p ck b u", ck=CK, b=B - 1
            )
            e0view = cs0[:, E0_OFF : E0_OFF + CK * 2 * BT].rearrange(
                "p (ck t) -> p ck t", ck=CK
            )
            p0view = cs0[:, P0_OFF : P0_OFF + CK * U].rearrange(
                "p (ck u) -> p ck u", ck=CK
            )
            bias2_f = bias_f32.unsqueeze(1).broadcast_to([P, 2, V])

            for b in range(B):
                for blk in range(BLK):
                    t0 = blk * BT
                    c0 = blk * BCELLS
                    bi = b * BLK + blk
                    kind = _BLOCK_ENG[bi]
                    lgt = logit_pool.tile([P, CK, BT, U], BF16, tag="lg")
                    # block 0 / batch 0 read from the small early blob; the
                    # rest from the big one
                    e_src = (
                        e0view[:, :, t0 : t0 + BT]
                        if bi < 2
                        else eview[:, :, b, t0 : t0 + BT]
                    )
                    p_src = p0view[:, :, :] if b == 0 else pview[:, :, b - 1, :]
                    e_all = e_src.unsqueeze(3).broadcast_to([P, CK, BT, U])
                    p_all = p_src.unsqueeze(2).broadcast_to([P, CK, BT, U])
                    if kind == "S":
                        # startup: spread blocks 0-1 across DVE+GPSIMD per-ck
                        for ck in range(CK):
                            arg = arg_pool.tile([P, BT, U], F32, tag=f"a0{ck}")
                            eng = nc.vector if (ck + bi) % 2 == 0 else nc.gpsimd
                            eng.tensor_add(
                                out=arg[:], in0=e_all[:, ck], in1=p_all[:, ck]
                            )
                            nc.scalar.activation(
                                out=lgt[:, ck],
                                in_=arg[:],
                                func=mybir.ActivationFunctionType.Tanh,
                            )
                    elif kind == "A":
                        # fused add+tanh on ACT, one op per (ck, t)
                        for ck in range(CK):
                            for t in range(BT):
                                nc.scalar.activation(
                                    out=lgt[:, ck, t, :],
                                    in_=p_src[:, ck, :],
                                    func=mybir.ActivationFunctionType.Tanh,
                                    bias=e_src[:, ck, t : t + 1],
                                )
                    else:
                        # one whole-block add + tanh (FD=4000 amortizes the
                        # per-op overhead and pipe-drain)
                        arg = arg_pool.tile([P, CK, BT, U], F32, tag="arg")
                        eng = nc.vector if kind == "D" else nc.gpsimd
                        eng.tensor_add(out=arg[:], in0=e_all, in1=p_all)
                        nc.scalar.activation(
                            out=lgt[:],
                            in_=arg[:],
                            func=mybir.ActivationFunctionType.Tanh,
                        )
                    lg = lgt[:].rearrange("p ck t u -> p ck (t u)")
                    # one osb mega-tile + one DMA per block keeps the sync
                    # HWDGE queue at ~20 ops instead of 160
                    osb = out_pool.tile([P, NT, V], BF16, tag="osb")
                    for q in range(NT // 2):
                        ps = psum_pool.tile([P, 2, V], F32, tag="ps")
                        for j in range(2):
                            s = (q * 2 + j) * TILE
                            for ck in range(CK):
                                nc.tensor.matmul(
                                    ps[:TILE, j, :],
                                    lhsT=lg[:, ck, s : s + TILE],
                                    rhs=wview[:, ck, :],
                                    start=(ck == 0),
                                    stop=(ck == CK - 1),
                                )
                        nc.vector.tensor_add(
                            out=osb[:TILE, 2 * q : 2 * q + 2, :],
                            in0=ps[:TILE],
                            in1=bias2_f[:TILE],
                        )
                    nc.sync.dma_start(
                        out=out.ap()[b, :, blk], in_=osb[:TILE]
                    )
    nc.compile()
    return nc


def _install_ntff_hook():
    """This image's antenv lacks axon_hooks, so bass_utils' trace=True path
    can't find the NTFF profile hook. Inject the module and wire the ctypes
    hook from trn_boot against the axon PJRT .so."""
    if "antenv.axon_hooks" in sys.modules:
        return
    import types

    holder = [None]
    mod = types.ModuleType("antenv.axon_hooks")
    mod.set_axon_ntff_profile_hook = lambda h: holder.__setitem__(0, h)
    mod.get_axon_ntff_profile_hook = lambda: holder[0]
    sys.modules["antenv.axon_hooks"] = mod
    try:
        sys.path.insert(0, "/root/.axon_site/trn_agent_boot")
        from trn_boot import _ntff_profile_via_ctypes

        mod.set_axon_ntff_profile_hook(
            _ntff_profile_via_ctypes("/opt/axon/libaxon_pjrt.so")
        )
    except Exception as e:  # degrade to no tracing
        print(f"NTFF hook install failed: {e}", file=sys.stderr)


def _run(in_maps, trace=False, tmpdir=None):
    if "nc" not in _cache:
        _cache["nc"] = _build()
    if trace:
        _install_ntff_hook()
    return run_bass_kernel_spmd(
        _cache["nc"], in_maps, list(range(NCORES)), trace=trace, tmpdir=tmpdir
    )


def make_in_maps(encoder_out, predictor_out, W, b):
    encoder_out = np.asarray(encoder_out, dtype=np.float32)
    predictor_out = np.asarray(predictor_out, dtype=np.float32)
    W = np.asarray(W, dtype=np.float32)
    b = np.asarray(b, dtype=np.float32)

    # [p, ck, v] <- W[v, ck*P+p]
    wpack = np.ascontiguousarray(
        W.reshape(V, CK, P).transpose(2, 1, 0).reshape(P, CK * V)
    ).astype(ml_dtypes.bfloat16)

    base = np.empty((P, NCOL_R), np.float32)
    base[:, BIAS_OFF : BIAS_OFF + V] = np.broadcast_to(b, (P, V))
    # [p, ck, b-1, u] <- pred[b, u, ck*P+p] for b = 1..3
    base[:, PRED_OFF : PRED_OFF + CK * (B - 1) * U] = (
        predictor_out[1:].reshape(B - 1, U, CK, P).transpose(3, 2, 0, 1).reshape(P, -1)
    )
    # [p, ck, u] <- pred[0, u, ck*P+p]
    p0 = predictor_out[0].reshape(U, CK, P).transpose(2, 1, 0).reshape(P, -1)

    in_maps = []
    for i in range(NCORES):
        m = base.copy()
        enc_s = encoder_out[:, i * TS : (i + 1) * TS, :]  # [b, t, c]
        m[:, ENC_OFF : ENC_OFF + CK * B * TS] = (
            enc_s.reshape(B, TS, CK, P).transpose(3, 2, 0, 1).reshape(P, -1)
        )
        m0 = np.empty((P, NCOL_0), np.float32)
        # [p, ck, t] <- enc_s[0, t, ck*P+p] for t in blocks 0-1
        m0[:, E0_OFF : E0_OFF + CK * 2 * BT] = (
            enc_s[0, : 2 * BT].reshape(2 * BT, CK, P).transpose(2, 1, 0).reshape(P, -1)
        )
        m0[:, P0_OFF : P0_OFF + CK * U] = p0
        in_maps.append({"cf0": m0, "cfr": m, "cw": wpack})
    return in_maps


def _unpack_out(arr):
    # [B, TILE(p), BLK, NT(j), V] -> [B, TS, U, V]; block cell = j*TILE + p
    a = np.asarray(arr).transpose(0, 2, 3, 1, 4).astype(np.float32)
    return a.reshape(B, TS, U, V)


def kernel(encoder_out, predictor_out, W, b):
    in_maps = make_in_maps(encoder_out, predictor_out, W, b)
    res = _run(in_maps, trace=False)
    return np.concatenate(
        [_unpack_out(res.results[i]["out"]) for i in range(NCORES)], axis=1
    )


# revision 19
# speedup vs baseline: 1.1337x; 1.1337x over previous
"""RNN-T Joiner kernel for 8 Trainium2 NeuronCores.

out[b,t,u,:] = tanh(enc[b,t,:] + pred[b,u,:]) @ W.T + b

Sharding: data-parallel over t (400 -> 50 per core). Each core:
  - DVE/GPSIMD: broadcast-add encT[:,t] + predT[:,u] (f32 -> f32; bf16
    outputs would halve the DVE tensor_tensor rate)
  - ACT: tanh f32 -> bf16 logit (the cast rides the activation)
  - PE:  psum[cells, v] += logitT[c, cells].T @ WT[c, v]  (bf16, N=512)
  - output pass per 4-tile psum quad (FD=2048 amortizes the 120-cyc
    PSUM read bubble), two alternating paths so ACT shares the load:
      1-pass: DVE tensor_tensor psum(f32)+bias -> osb bf16   (~2.4us)
      2-pass: ACT quad copy psum->osb bf16 (~1.9us), DVE bf16+bias4_bf
              in place (2x_1P, ~1.2us)
  - DMA: one 500-cell (512KB) store per quad on the sync HWDGE queue
  - host: upcast bf16 -> f32 during the gather

Engine budget per core (cycle-model + measured): PE 640 MM x ~216ns =
~138us critical path; DVE ~116us (20 one-pass quads, 20 bf16 bias adds,
38 producer adds @1.2us), GPSIMD ~97us (42 adds @2.3us), ACT ~119us
(80 tanh @1.02us + 20 quad copies @1.85us), DMA out bf16 20.5MB ~50us.
BLK=5 keeps the startup serial chain (consts DMA -> add -> tanh) ~8us.
"""

import sys

sys.path.insert(0, "/opt/trn_rl_repo")

import ml_dtypes
import numpy as np

import concourse.bass as bass
import concourse.bacc as bacc
import concourse.mybir as mybir
from concourse.tile import TileContext
from concourse.bass_utils import run_bass_kernel_spmd

B, T, U, C, V = 4, 400, 100, 512, 512
NCORES = 8
TS = T // NCORES  # 50 t per core
P = 128
CK = C // P  # 4 chunks of the contraction dim
CELLS = TS * U  # 5000 cells (t,u) per batch per core
BLK = 5  # logit blocks per batch
BCELLS = CELLS // BLK  # 1000
BT = TS // BLK  # 10 t per block
TILE = 125  # uniform cell tile (8 per block); m<128 costs no PE streaming
NT = BCELLS // TILE  # 8
NQ = NT // 2  # tiles per psum quad... actually 4 tiles per quad, 2 quads
F32 = mybir.dt.float32
BF16 = mybir.dt.bfloat16

# f32 consts blobs. cf0 is tiny (block-0 enc slice + batch-0 pred) so the
# first producer isn't gated on the big transfer; cfr has the rest.
E0_OFF = 0  # [ck, t] for blocks 0-1 (t = 0..19)
P0_OFF = E0_OFF + CK * 2 * BT  # [ck, u]
NCOL_0 = P0_OFF + CK * U  # 480
BIAS_OFF = 0
ENC_OFF = BIAS_OFF + V  # [ck, b, t] (all b)
PRED_OFF = ENC_OFF + CK * B * TS  # [ck, b-1, u] (b = 1..3)
NCOL_R = PRED_OFF + CK * (B - 1) * U  # 2512
# bf16 consts blob: just W, pre-transposed [p, ck, v]
NCOL_W = CK * V  # 2048

N_BLOCKS = B * BLK  # 20

# producer engine per block (one [P, 4*BT*U] mega add+tanh per block):
# D=DVE add + ACT tanh, G=GPSIMD add + ACT tanh, A=ACT fused per-t ops.
# Block 0 is additionally split per-ck across DVE/GPSIMD so the first
# matmul isn't gated on one engine's serial 4000-col chain.
# S = blocks 0-1 (split per-ck across DVE+GPSIMD for startup). No ACT-fused
# blocks: they shift 7.4us of GPSIMD work into ~11us of serial ACT work and
# GPSIMD is not the constraint.
_BLOCK_ENG = list("SSDGG" "GGDGG" "GDGGG" "GGDGG")


_cache = {}


def _build():
    # Bacc (not raw Bass): its compile() runs generate_event_semaphores,
    # which splits >1-wait sync conditions that walrus rejects.
    nc = bacc.Bacc("TRN2", target_bir_lowering=False, debug=False)
    cf0 = nc.declare_dram_parameter("cf0", [P, NCOL_0], F32, isOutput=False)
    cfr = nc.declare_dram_parameter("cfr", [P, NCOL_R], F32, isOutput=False)
    cw = nc.declare_dram_parameter("cw", [P, NCOL_W], BF16, isOutput=False)
    # block-major output layout: [b, blk, p, j, v], cell = j*TILE + p for
    # p < TILE; rows TILE..127 are pad. The DMA must cover all 128 source
    # partitions: <128-partition stores fall into a 5-of-16-SDMA-engine
    # fallback (~130GB/s ceiling) while 128-partition stores spread across
    # all 16 engines (~400GB/s). Host slices the pad off during the upcast.
    out = nc.declare_dram_parameter(
        "out", [B, BLK, P, NT, V], BF16, isOutput=True
    )

    with TileContext(nc) as tc:
        with (
            tc.tile_pool(name="consts", bufs=1) as cpool,
            tc.tile_pool(name="arg", bufs=4) as arg_pool,
            tc.tile_pool(name="logit", bufs=4) as logit_pool,
            tc.tile_pool(name="osb", bufs=3) as out_pool,
            tc.tile_pool(name="psum", bufs=4, space="PSUM") as psum_pool,
        ):
            cs0 = cpool.tile([P, NCOL_0], F32, tag="cs0")
            csf = cpool.tile([P, NCOL_R], F32, tag="csf")
            csw = cpool.tile([P, NCOL_W], BF16, tag="csw")
            nc.sync.dma_start(out=cs0, in_=cf0.ap())
            nc.sync.dma_start(out=csw, in_=cw.ap())
            nc.sync.dma_start(out=csf, in_=cfr.ap())

            wview = csw[:].rearrange("p (ck v) -> p ck v", ck=CK)
            bias_f32 = csf[:, BIAS_OFF : BIAS_OFF + V]
            eview = csf[:, ENC_OFF : ENC_OFF + CK * B * TS].rearrange(
                "p (ck b t) -> p ck b t", ck=CK, b=B
            )
            pview = csf[:, PRED_OFF : PRED_OFF + CK * (B - 1) * U].rearrange(
                "p (ck b u) -> p ck b u", ck=CK, b=B - 1
            )
            e0view = cs0[:, E0_OFF : E0_OFF + CK * 2 * BT].rearrange(
                "p (ck t) -> p ck t", ck=CK
            )
            p0view = cs0[:, P0_OFF : P0_OFF + CK * U].rearrange(
                "p (ck u) -> p ck u", ck=CK
            )
            bias2_f = bias_f32.unsqueeze(1).broadcast_to([P, 2, V])

            for b in range(B):
                for blk in range(BLK):
                    t0 = blk * BT
                    c0 = blk * BCELLS
                    bi = b * BLK + blk
                    kind = _BLOCK_ENG[bi]
                    lgt = logit_pool.tile([P, CK, BT, U], BF16, tag="lg")
                    # block 0 / batch 0 read from the small early blob; the
                    # rest from the big one
                    e_src = (
                        e0view[:, :, t0 : t0 + BT]
                        if bi < 2
                        else eview[:, :, b, t0 : t0 + BT]
                    )
                    p_src = p0view[:, :, :] if b == 0 else pview[:, :, b - 1, :]
                    e_all = e_src.unsqueeze(3).broadcast_to([P, CK, BT, U])
                    p_all = p_src.unsqueeze(2).broadcast_to([P, CK, BT, U])
                    if kind == "S":
                        # startup: spread blocks 0-1 across DVE+GPSIMD per-ck
                        for ck in range(CK):
                            arg = arg_pool.tile([P, BT, U], F32, tag=f"a0{ck}")
                            eng = nc.vector if (ck + bi) % 2 == 0 else nc.gpsimd
                            eng.tensor_add(
                                out=arg[:], in0=e_all[:, ck], in1=p_all[:, ck]
                            )
                            nc.scalar.activation(
                                out=lgt[:, ck],
                                in_=arg[:],
                                func=mybir.ActivationFunctionType.Tanh,
                            )
                    elif kind == "A":
                        # fused add+tanh on ACT, one op per (ck, t)
                        for ck in range(CK):
                            for t in range(BT):
                                nc.scalar.activation(
                                    out=lgt[:, ck, t, :],
                                    in_=p_src[:, ck, :],
                                    func=mybir.ActivationFunctionType.Tanh,
                                    bias=e_src[:, ck, t : t + 1],
                                )
                    else:
                        # one whole-block add + tanh (FD=4000 amortizes the
                        # per-op overhead and pipe-drain)
                        arg = arg_pool.tile([P, CK, BT, U], F32, tag="arg")
                        eng = nc.vector if kind == "D" else nc.gpsimd
                        eng.tensor_add(out=arg[:], in0=e_all, in1=p_all)
                        nc.scalar.activation(
                            out=lgt[:],
                            in_=arg[:],
                            func=mybir.ActivationFunctionType.Tanh,
                        )
                    lg = lgt[:].rearrange("p ck t u -> p ck (t u)")
                    # one osb mega-tile + one DMA per block keeps the sync
                    # HWDGE queue at ~20 ops instead of 160
                    osb = out_pool.tile([P, NT, V], BF16, tag="osb")
                    for q in range(NT // 2):
                        ps = psum_pool.tile([P, 2, V], F32, tag="ps")
                        for j in range(2):
                            s = (q * 2 + j) * TILE
                            for ck in range(CK):
                                nc.tensor.matmul(
                                    ps[:TILE, j, :],
                                    lhsT=lg[:, ck, s : s + TILE],
                                    rhs=wview[:, ck, :],
                                    start=(ck == 0),
                                    stop=(ck == CK - 1),
                                )
                        nc.vector.tensor_add(
                            out=osb[:TILE, 2 * q : 2 * q + 2, :],
                            in0=ps[:TILE],
                            in1=bias2_f[:TILE],
                        )
                    nc.sync.dma_start(out=out.ap()[b, blk], in_=osb[:])
    nc.compile()
    return nc


def _install_ntff_hook():
    """This image's antenv lacks axon_hooks, so bass_utils' trace=True path
    can't find the NTFF profile hook. Inject the module and wire the ctypes
    hook from trn_boot against the axon PJRT .so."""
    if "antenv.axon_hooks" in sys.modules:
        return
    import types

    holder = [None]
    mod = types.ModuleType("antenv.axon_hooks")
    mod.set_axon_ntff_profile_hook = lambda h: holder.__setitem__(0, h)
    mod.get_axon_ntff_profile_hook = lambda: holder[0]
    sys.modules["antenv.axon_hooks"] = mod
    try:
        sys.path.insert(0, "/root/.axon_site/trn_agent_boot")
        from trn_boot import _ntff_profile_via_ctypes

        mod.set_axon_ntff_profile_hook(
            _ntff_profile_via_ctypes("/opt/axon/libaxon_pjrt.so")
        )
    except Exception as e:  # degrade to no tracing
        print(f"NTFF hook install failed: {e}", file=sys.stderr)


def _run(in_maps, trace=False, tmpdir=None):
    if "nc" not in _cache:
        _cache["nc"] = _build()
    if trace:
        _install_ntff_hook()
    return run_bass_kernel_spmd(
        _cache["nc"], in_maps, list(range(NCORES)), trace=trace, tmpdir=tmpdir
    )


def make_in_maps(encoder_out, predictor_out, W, b):
    encoder_out = np.asarray(encoder_out, dtype=np.float32)
    predictor_out = np.asarray(predictor_out, dtype=np.float32)
    W = np.asarray(W, dtype=np.float32)
    b = np.asarray(b, dtype=np.float32)

    # [p, ck, v] <- W[v, ck*P+p]
    wpack = np.ascontiguousarray(
        W.reshape(V, CK, P).transpose(2, 1, 0).reshape(P, CK * V)
    ).astype(ml_dtypes.bfloat16)

    base = np.empty((P, NCOL_R), np.float32)
    base[:, BIAS_OFF : BIAS_OFF + V] = np.broadcast_to(b, (P, V))
    # [p, ck, b-1, u] <- pred[b, u, ck*P+p] for b = 1..3
    base[:, PRED_OFF : PRED_OFF + CK * (B - 1) * U] = (
        predictor_out[1:].reshape(B - 1, U, CK, P).transpose(3, 2, 0, 1).reshape(P, -1)
    )
    # [p, ck, u] <- pred[0, u, ck*P+p]
    p0 = predictor_out[0].reshape(U, CK, P).transpose(2, 1, 0).reshape(P, -1)

    in_maps = []
    for i in range(NCORES):
        m = base.copy()
        enc_s = encoder_out[:, i * TS : (i + 1) * TS, :]  # [b, t, c]
        m[:, ENC_OFF : ENC_OFF + CK * B * TS] = (
            enc_s.reshape(B, TS, CK, P).transpose(3, 2, 0, 1).reshape(P, -1)
        )
        m0 = np.empty((P, NCOL_0), np.float32)
        # [p, ck, t] <- enc_s[0, t, ck*P+p] for t in blocks 0-1
        m0[:, E0_OFF : E0_OFF + CK * 2 * BT] = (
            enc_s[0, : 2 * BT].reshape(2 * BT, CK, P).transpose(2, 1, 0).reshape(P, -1)
        )
        m0[:, P0_OFF : P0_OFF + CK * U] = p0
        in_maps.append({"cf0": m0, "cfr": m, "cw": wpack})
    return in_maps


def _unpack_out(arr):
    # [B, BLK, P(pad), NT(j), V] -> [B, TS, U, V]; block cell = j*TILE + p
    a = np.asarray(arr)[:, :, :TILE].transpose(0, 1, 3, 2, 4)
    return a.astype(np.float32).reshape(B, TS, U, V)


def kernel(encoder_out, predictor_out, W, b):
    in_maps = make_in_maps(encoder_out, predictor_out, W, b)
    res = _run(in_maps, trace=False)
    return np.concatenate(
        [_unpack_out(res.results[i]["out"]) for i in range(NCORES)], axis=1
    )
